# revision 41
# baseline (speedup 1.0000x reference)
"""Dual-stream fused attention kernel for 8 TRN2 NeuronCores.

Reference computation (B=2, N=2048, D=512, H=8, Dh=64):
    qkv_s = x_s @ W_qkv_s (s = 1,2)  -> per-head q_s, k_s, v_s
    dots  = SCALE * (q1 k1^T + q2 k2^T)          [b, h, n, n]
    attn  = softmax(dots)
    out_s = attn @ v_s                           [b, h, n, dh]
    out   = concat(merge(out1), merge(out2), axis=1) @ W_out + b_out

Sharding: core c handles batch b = c//4 and heads {2*(c%4), 2*(c%4)+1}
(data parallel on b, tensor parallel on h). Each core computes a partial
out-projection over its 128 inner columns; the host sums the 4 partials
per batch (the TP all-reduce) and adds b_out.

On-core dataflow (all matmuls bf16, fp32 PSUM accumulation; every
PSUM-reading instruction is 512 wide -- wider reads cross PSUM banks
and run slower per element):
  - QK projections run at full array efficiency (M=128: both heads at
    once, per stream); the (stream-stacked per head) QT/KT layout the
    score matmuls need is then assembled by SBUF->SBUF DMAs on the
    otherwise-idle DMA queues.
  - V is projected transposed (stationary weights, wide moving x), then
    PE-transposed back per key block, so the projection matmuls are few
    and wide instead of 128 narrow stationary-switching ones.
  - Scores are computed transposed, S^T [k, q], so exp needs no
    transpose and P^T feeds the AV matmul directly as moving operand.
  - Softmax is max-free (|SCALE * dots| <~ 1.5 for this problem's data
    distribution, exp cannot overflow); the denominator is accumulated
    on the vector engine and reduced across partitions via ones-matmuls.
  - 1/rowsum is computed as exp(-ln(rowsum)) on the scalar engine (ln
    and exp share one ACT table set; DVE reciprocal is ~8 cycles/elem).
  - Normalization is fused into the AV-PSUM evacuation; merged per-head
    outputs feed the output projection as its stationary operand.
  - Emission is software-pipelined: scores lead one k-block so the PE
    never sits behind the exp stream in queue order, and out-projection
    matmuls of a finished q-block are dripped into the next block's
    PE stream as filler (keeps the PE busy so the HAM clock gate stays
    at 2.4 GHz).
"""

import numpy as np
import ml_dtypes

import bass_rust
import concourse.bass as bass
import concourse.mybir as mybir
import concourse.tile as tile
from concourse.vector_clock import ScopedClock
from concourse.bass_utils import run_bass_kernel_spmd

B, N, D = 2, 2048, 512
H, DH = 8, 64
SCALE = (2 * DH) ** -0.5
NCORES = 8
HPC = 2              # heads per core
CW = HPC * DH        # 128: per-core slice width of the inner dim
DC = D // 128        # 4 contraction chunks for the projections
NKB = N // 128       # 16 key blocks
QB = 1024            # q-block width for the attention inner loop
NQB = N // QB        # 2
BF16 = ml_dtypes.bfloat16

_WAIT_LIMIT = 1  # this container's walrus rejects multiple sync waits per instruction


def _split_sync_waits(nc):
    """Hoist excess semaphore waits onto same-engine NOPs inserted right
    before the over-budget instruction ("Too many sync wait commands")."""
    for f in nc.m.functions:
        for bb in f.blocks:
            insts = bb.instructions
            i = 0
            while i < len(insts):
                inst = insts[i]
                si = inst.sync_info
                if si is None:
                    i += 1
                    continue
                waits = list(si.on_wait)
                sem_waits = [w for w in waits if w.sync_type == "semaphore"]
                other = [w for w in waits if w.sync_type != "semaphore"]
                budget = _WAIT_LIMIT - len(other)
                if len(sem_waits) <= budget:
                    i += 1
                    continue
                keep = sem_waits[-budget:] if budget > 0 else []
                extra = sem_waits[:-budget] if budget > 0 else sem_waits
                for j in range(0, len(extra), _WAIT_LIMIT):
                    nop = mybir.InstNoOp(
                        name=f"I-{nc.next_id()}",
                        engine=inst.engine,
                        bass_nofuse=True,
                        sync_info=mybir.SyncInfo(
                            on_wait=extra[j:j + _WAIT_LIMIT], on_update=[]
                        ),
                    )
                    insts.insert(i, nop)
                    i += 1
                si.on_wait = other + keep
                inst.sync_info = si
                i += 1


def _body(nc, tc):
    bf = mybir.dt.bfloat16
    f32 = mybir.dt.float32
    EXP = mybir.ActivationFunctionType.Exp
    LN = mybir.ActivationFunctionType.Ln

    x1T = nc.dram_tensor("x1T", [D, N], bf, kind="ExternalInput").ap()
    x2T = nc.dram_tensor("x2T", [D, N], bf, kind="ExternalInput").ap()
    # weights come pre-rearranged from the host as [p, dc*c] so the load
    # streams contiguous 1KB lines instead of 256B gather lines
    wq = [nc.dram_tensor(f"wq{s}", [128, DC * CW], bf, kind="ExternalInput").ap() for s in (1, 2)]
    wk = [nc.dram_tensor(f"wk{s}", [128, DC * CW], bf, kind="ExternalInput").ap() for s in (1, 2)]
    wv = [nc.dram_tensor(f"wv{s}", [128, DC * CW], bf, kind="ExternalInput").ap() for s in (1, 2)]
    wout = nc.dram_tensor("wout", [CW, D], bf, kind="ExternalInput").ap()
    ident = nc.dram_tensor("ident", [128, 128], bf, kind="ExternalInput").ap()
    out = nc.dram_tensor("out", [2 * N, D], bf, kind="ExternalOutput").ap()
    xT = [x1T, x2T]

    pools = []

    def mkpool(**kw):
        p = tc.alloc_tile_pool(**kw)
        pools.append(p)
        return p

    singles = mkpool(name="singles", bufs=1)
    spool = mkpool(name="spool", bufs=4, space="PSUM")      # 4x [128,512]f32 = 4 banks
    avpool = mkpool(name="avpool", bufs=2, space="PSUM")    # 2x [128,1024]f32 = 4 banks
    ptpool = mkpool(name="ptpool", bufs=8)
    accpool = mkpool(name="accpool", bufs=2)
    smallpool = mkpool(name="smallpool", bufs=3)
    stagepool = mkpool(name="stagepool", bufs=4)
    ostage = mkpool(name="ostage", bufs=8)

    # ---- resident inputs -------------------------------------------------
    # Load order matters for the PE start: qk weights, then x quarter 0,
    # then the rest, spread over the three DMA-trigger queues.
    dma_engines = [nc.sync, nc.scalar, nc.gpsimd]

    def load_w(ap, name, eng):
        t = singles.tile([128, DC, CW], bf, tag=name, name=name)
        eng.dma_start(out=t, in_=ap.rearrange("p (dc c) -> p dc c", dc=DC))
        return t

    x_sb = [[singles.tile([128, N], bf, tag=f"x{s}_{dc}", name=f"x{s}_{dc}")
             for dc in range(DC)] for s in range(2)]

    def load_x_half(half):
        n0, n1 = half * (N // 2), (half + 1) * (N // 2)
        for s in range(2):
            for dc in range(DC):
                eng = dma_engines[(s * DC + dc) % len(dma_engines)]
                eng.dma_start(out=x_sb[s][dc][:, n0:n1],
                              in_=xT[s][dc * 128:(dc + 1) * 128, n0:n1])

    load_x_half(0)
    wq_sb = [load_w(wq[s], f"wq{s}", dma_engines[s]) for s in range(2)]
    wk_sb = [load_w(wk[s], f"wk{s}", dma_engines[(2 + s) % 3]) for s in range(2)]
    wv_sb = [load_w(wv[s], f"wv{s}", dma_engines[s]) for s in range(2)]
    load_x_half(1)

    ones_mat = singles.tile([128, 128], bf, tag="ones", name="ones")
    nc.vector.memset(ones_mat, 1.0)
    ident_sb = singles.tile([128, 128], bf, tag="ident", name="ident")
    nc.gpsimd.dma_start(out=ident_sb, in_=ident)
    wout_sb = singles.tile([CW, D], bf, tag="wout", name="wout")
    nc.scalar.dma_start(out=wout_sb, in_=wout)

    # persistent layout targets
    qt = [singles.tile([128, N], bf, tag=f"qt{h}", name=f"qt{h}") for h in range(HPC)]
    kt = [singles.tile([128, N], bf, tag=f"kt{h}", name=f"kt{h}") for h in range(HPC)]
    vt_sb = [singles.tile([128, N], bf, tag=f"vt{s}", name=f"vt{s}") for s in range(2)]
    v_all = singles.tile([128, NKB, HPC, 2, DH], bf, tag="vall", name="vall")
    merged = [singles.tile([128, N], bf, tag=f"merged{s}", name=f"merged{s}") for s in range(2)]

    n_dma = [0]

    def next_dma():
        e = dma_engines[n_dma[0] % len(dma_engines)]
        n_dma[0] += 1
        return e

    # ---- projections, 512-wide psum units over both pools ----------------
    # spool gives four 1-bank slots during the projection phase (attention
    # hasn't started), so the PE never stalls on evacuation; evac copies
    # alternate scalar/vector.
    n_evac = [0]

    def evac(out_ap, in_ap, eng=None):
        if eng is None:
            eng = nc.scalar if n_evac[0] % 2 == 0 else nc.vector
            n_evac[0] += 1
        if eng is nc.scalar:
            nc.scalar.copy(out=out_ap, in_=in_ap)
        else:
            eng.tensor_copy(out=out_ap, in_=in_ap)

    def proj_qk_unit(nch, dst, w_sb, s, eng=None):
        """One (dst, stream) 1024-col chunk of the QK projection: M=128
        matmuls (both heads at once); the per-head stream-stacked layout
        is then assembled by SBUF->SBUF DMA."""
        c0 = nch * 1024
        st = stagepool.tile([128, 1024], bf, tag="qkstage", name="qkstage")
        for half in range(2):
            h0 = c0 + half * 512
            ps = spool.tile([128, 512], f32, tag="s", name="qkps")
            for dc in range(DC):
                nc.tensor.matmul(
                    ps,
                    lhsT=w_sb[s][:, dc, :],
                    rhs=x_sb[s][dc][:, h0:h0 + 512],
                    start=(dc == 0),
                    stop=(dc == DC - 1),
                )
            evac(st[:, half * 512:(half + 1) * 512], ps, eng)
        for h in range(HPC):
            next_dma().dma_start(
                out=dst[h][s * 64:(s + 1) * 64, c0:c0 + 1024],
                in_=st[h * 64:(h + 1) * 64, :],
            )

    def proj_qk_chunk(nch):
        for dst, w_sb in ((qt, wq_sb), (kt, wk_sb)):
            for s in range(2):
                proj_qk_unit(nch, dst, w_sb, s)

    def proj_vt_unit(nch, s, eng=None):
        c0 = nch * 1024
        for half in range(2):
            h0 = c0 + half * 512
            ps = spool.tile([128, 512], f32, tag="s", name="vtps")
            for dc in range(DC):
                nc.tensor.matmul(
                    ps,
                    lhsT=wv_sb[s][:, dc, :],
                    rhs=x_sb[s][dc][:, h0:h0 + 512],
                    start=(dc == 0),
                    stop=(dc == DC - 1),
                )
            evac(vt_sb[s][:, h0:h0 + 512], ps, eng)

    def proj_v_transpose(nch):
        """PE-transpose each key block of VT back into v_all natural
        layout [k, (kb, h, s, dh)]; both streams share one psum tile."""
        vps = avpool.tile([128, 2048], bf, tag="av", name="vps")
        kb0 = nch * 8
        for s in range(2):
            for i in range(8):
                nc.tensor.transpose(
                    vps[:, s * 1024 + i * 128:s * 1024 + (i + 1) * 128],
                    vt_sb[s][:, (kb0 + i) * 128:(kb0 + i + 1) * 128],
                    ident_sb,
                )
        for s in range(2):
            nc.vector.tensor_copy(
                out=v_all[:, kb0:kb0 + 8, :, s, :],
                in_=vps[:, s * 1024:(s + 1) * 1024].rearrange(
                    "p (i h d) -> p i h d", i=8, h=HPC),
            )

    def proj_v_chunk(nch, eng=None):
        for s in range(2):
            proj_vt_unit(nch, s, eng)
        proj_v_transpose(nch)

    # ---- out-projection ops (emitted as PE filler or at the tail) --------
    def op_emit(s, rb, cast_eng, alt_pool=False):
        if alt_pool:
            op = avpool.tile([128, 512], f32, tag="av", name="op")
        else:
            op = spool.tile([128, 512], f32, tag="s", name="op")
        nc.tensor.matmul(
            op,
            lhsT=merged[s][:, rb * 128:(rb + 1) * 128],
            rhs=wout_sb,
            start=True, stop=True,
        )
        ost = ostage.tile([128, 512], bf, tag="ost", name="ost")
        if cast_eng is nc.scalar:
            cast_eng.copy(out=ost, in_=op)
        else:
            cast_eng.tensor_copy(out=ost, in_=op)
        next_dma().dma_start(
            out=out[s * N + rb * 128:s * N + (rb + 1) * 128, :],
            in_=ost,
        )

    def outproj_filler(qb):
        """The 16 out-projection row blocks of q-block qb as a list of
        thunks, to be dripped into a later attention block's emission."""
        return [
            (lambda s=s, rb=qb * (QB // 128) + j: op_emit(s, rb, nc.vector))
            for j in range(QB // 128) for s in range(2)
        ]

    # ---- attention -------------------------------------------------------
    def score_mm(h, q0, kb):
        tiles = []
        for qh in range(2):
            st = spool.tile([128, 512], f32, tag="s", name="s")
            nc.tensor.matmul(
                st,
                lhsT=kt[h][:, kb * 128:(kb + 1) * 128],
                rhs=qt[h][:, q0 + qh * 512:q0 + (qh + 1) * 512],
                start=True, stop=True,
            )
            tiles.append(st)
        return tiles

    def attn_block(qb, h, filler, inserts=None, warm_tail=False):
        """Score/exp/AV loop; scores lead one kb; filler thunks are
        popped every other kb (out-projection of an earlier q-block);
        inserts maps kb -> thunk emitted after that kb (late projection
        work dripped into the PE stream while exp paces the loop).
        The denominator add-chain stops two k-blocks early: the last two
        pt tiles enter the ones-matmul reduction directly via PSUM
        accumulation, shortening the tail latency before ln/exp."""
        q0 = qb * QB
        av = avpool.tile([128, 1024], f32, tag="av", name="av")
        acc = accpool.tile([128, 1024], bf, tag="acc", name="acc")
        bcs = []
        st = score_mm(h, q0, 0)
        for kb in range(NKB):
            if inserts and kb in inserts:
                inserts[kb]()
            pt = ptpool.tile([128, 1024], bf, tag="pt", name="pt")
            for qh in range(2):
                nc.scalar.activation(
                    out=pt[:, qh * 512:(qh + 1) * 512], in_=st[qh],
                    func=EXP, scale=SCALE,
                )
            if kb < NKB - 1:
                st = score_mm(h, q0, kb + 1)
            for qh in range(2):
                nc.tensor.matmul(
                    av[:, qh * 512:(qh + 1) * 512],
                    lhsT=v_all[:, kb, h, :, :],
                    rhs=pt[:, qh * 512:(qh + 1) * 512],
                    start=(kb == 0), stop=(kb == NKB - 1),
                )
            if kb == NKB - 2:
                # partition-reduce acc (kb 0..13) and this pt into bc
                for qh in range(2):
                    sl = slice(qh * 512, (qh + 1) * 512)
                    bc = spool.tile([128, 512], f32, tag="s", name="bc")
                    nc.tensor.matmul(bc, lhsT=ones_mat, rhs=acc[:, sl],
                                     start=True, stop=False)
                    nc.tensor.matmul(bc, lhsT=ones_mat, rhs=pt[:, sl],
                                     start=False, stop=False)
                    bcs.append(bc)
            elif kb == NKB - 1:
                for qh in range(2):
                    sl = slice(qh * 512, (qh + 1) * 512)
                    nc.tensor.matmul(bcs[qh], lhsT=ones_mat, rhs=pt[:, sl],
                                     start=False, stop=True)
            if filler and (kb % 2 == 1 or kb == 2) and kb < NKB - 1:
                filler.pop(0)()
            if kb == 0:
                nc.vector.tensor_copy(out=acc, in_=pt)
            elif kb < NKB - 2:
                nc.vector.tensor_add(out=acc, in0=acc, in1=pt)

        if warm_tail:
            # keep the PE busy across the fold-wait so the HAM clock gate
            # doesn't throttle it right before the tail out-projection
            for i in range(4):
                dmy = spool.tile([128, 512], f32, tag="s", name="dmy")
                nc.tensor.matmul(dmy, lhsT=warm_sb[:, 0:128], rhs=warm_sb,
                                 start=True, stop=True)

        # 1/D = exp(-ln(D)); normalization folded into AV evacuation
        rcp = smallpool.tile([128, 1024], f32, tag="rcp", name="rcp")
        for qh in range(2):
            sl = slice(qh * 512, (qh + 1) * 512)
            lnd = smallpool.tile([128, 512], f32, tag="lnd", name="lnd")
            nc.scalar.activation(out=lnd, in_=bcs[qh], func=LN)
            nc.scalar.activation(out=rcp[:, sl], in_=lnd, func=EXP, scale=-1.0)
        for s in range(2):
            for qh in range(2):
                sl = slice(qh * 512, (qh + 1) * 512)
                nc.vector.tensor_mul(
                    out=merged[s][h * 64:(h + 1) * 64,
                                  q0 + qh * 512:q0 + (qh + 1) * 512],
                    in0=av[s * 64:(s + 1) * 64, sl],
                    in1=rcp[s * 64:(s + 1) * 64, sl],
                )

    # ---- emission schedule ----------------------------------------------
    # Pre-warm the PE while the input DMAs are in flight: the HAM clock
    # gate starts at 1.2 GHz and needs sustained near-full PE activity to
    # release to 2.4 GHz. fp32 matmuls run 4 cycles/row (~94% duty per
    # instruction) and depend on nothing but a memset tile.
    warm_sb = singles.tile([128, 512], f32, tag="warm", name="warm")
    nc.vector.memset(warm_sb, 0.5)
    for _ in range(6):
        dmy = spool.tile([128, 512], f32, tag="s", name="dmy")
        nc.tensor.matmul(dmy, lhsT=warm_sb[:, 0:128], rhs=warm_sb,
                         start=True, stop=True)

    # Chunk-0 projections up front; chunk-1 projection units are dripped
    # into the first attention block's kb stream (their outputs are only
    # needed from kb 8 on). Evacs of dripped units go to the vector
    # engine so the exp stream on ACT is not disturbed.
    proj_qk_chunk(0)
    proj_v_chunk(0)

    proj_qk_chunk(1)
    proj_v_chunk(1)
    inserts00 = None
    inserts01 = None
    attn_block(0, 0, None, inserts00)
    # rotation spacer: keeps the next av accumulator off the slot of the
    # (not yet folded) previous one, so its first write doesn't stall
    spacer = avpool.tile([128, 1], f32, tag="av", name="spacer")
    nc.vector.memset(spacer, 0.0)
    attn_block(0, 1, None, inserts01)
    fill0 = outproj_filler(0)
    attn_block(1, 0, fill0)
    attn_block(1, 1, fill0, warm_tail=True)
    assert not fill0
    # tail out-projection: casts alternate vector/scalar (both idle now);
    # stream-major order so s=0 rows start as soon as their folds land
    for s in range(2):
        for j in range(QB // 128):
            op_emit(s, (QB // 128) + j, nc.vector if (j + s) % 2 else nc.scalar,
                    alt_pool=j % 2 == 0)

    for p in reversed(pools):
        p.release()


_NC_CACHE = None


def _build():
    global _NC_CACHE
    if _NC_CACHE is None:
        nc = bass.Bass("TRN2", target_bir_lowering=False, debug=False)
        with tile.TileContext(nc) as tc:
            _body(nc, tc)
        _split_sync_waits(nc)
        _NC_CACHE = nc
    return _NC_CACHE


def _prep_in_maps(x1, x2, W_qkv1, W_qkv2, W_out):
    x1 = np.asarray(x1, np.float32)
    x2 = np.asarray(x2, np.float32)
    W1 = np.asarray(W_qkv1, np.float32).astype(BF16)
    W2 = np.asarray(W_qkv2, np.float32).astype(BF16)
    Wo = np.asarray(W_out, np.float32).astype(BF16)
    ident = np.eye(128, dtype=BF16)
    xT = [
        [np.ascontiguousarray(x[b].T).astype(BF16) for b in range(B)]
        for x in (x1, x2)
    ]

    def wprep(w):
        # [D, CW] -> [p, dc*c]: contiguous per-partition lines for the DMA
        return np.ascontiguousarray(
            w.reshape(DC, 128, CW).transpose(1, 0, 2).reshape(128, DC * CW))

    in_maps = []
    for c in range(NCORES):
        b, hg = divmod(c, NCORES // B)
        cs = slice(hg * CW, (hg + 1) * CW)
        in_maps.append({
            "x1T": xT[0][b],
            "x2T": xT[1][b],
            "wq1": wprep(W1[:, 0:D][:, cs]),
            "wq2": wprep(W2[:, 0:D][:, cs]),
            "wk1": wprep(W1[:, D:2 * D][:, cs]),
            "wk2": wprep(W2[:, D:2 * D][:, cs]),
            "wv1": wprep(W1[:, 2 * D:3 * D][:, cs]),
            "wv2": wprep(W2[:, 2 * D:3 * D][:, cs]),
            "wout": np.ascontiguousarray(Wo[cs, :]),
            "ident": ident,
        })
    return in_maps


def _run(inputs, **spmd_kwargs):
    nc = _build()
    in_maps = _prep_in_maps(
        inputs["x1"], inputs["x2"], inputs["W_qkv1"], inputs["W_qkv2"],
        inputs["W_out"],
    )
    res = run_bass_kernel_spmd(nc, in_maps, core_ids=list(range(NCORES)),
                               **spmd_kwargs)
    b_out = np.asarray(inputs["b_out"], np.float32)
    gpc = NCORES // B
    full = np.zeros((B, 2 * N, D), np.float32)
    for c in range(NCORES):
        full[c // gpc] += res.results[c]["out"].astype(np.float32)
    full += b_out
    return full, res


def kernel(**inputs):
    full, _ = _run(inputs)
    return full


# revision 42
# speedup vs baseline: 1.0206x; 1.0206x over previous
"""Dual-stream fused attention kernel for 8 TRN2 NeuronCores.

Reference computation (B=2, N=2048, D=512, H=8, Dh=64):
    qkv_s = x_s @ W_qkv_s (s = 1,2)  -> per-head q_s, k_s, v_s
    dots  = SCALE * (q1 k1^T + q2 k2^T)          [b, h, n, n]
    attn  = softmax(dots)
    out_s = attn @ v_s                           [b, h, n, dh]
    out   = concat(merge(out1), merge(out2), axis=1) @ W_out + b_out

Sharding: core c handles batch b = c//4 and heads {2*(c%4), 2*(c%4)+1}
(data parallel on b, tensor parallel on h). Each core computes a partial
out-projection over its 128 inner columns; the host sums the 4 partials
per batch (the TP all-reduce) and adds b_out.

On-core dataflow (all matmuls bf16, fp32 PSUM accumulation; every
PSUM-reading instruction is 512 wide -- wider reads cross PSUM banks
and run slower per element):
  - QK projections run at full array efficiency (M=128: both heads at
    once, per stream); the (stream-stacked per head) QT/KT layout the
    score matmuls need is then assembled by SBUF->SBUF DMAs on the
    otherwise-idle DMA queues.
  - V is projected transposed (stationary weights, wide moving x), then
    PE-transposed back per key block, so the projection matmuls are few
    and wide instead of 128 narrow stationary-switching ones.
  - Scores are computed transposed, S^T [k, q], so exp needs no
    transpose and P^T feeds the AV matmul directly as moving operand.
  - Softmax is max-free (|SCALE * dots| <~ 1.5 for this problem's data
    distribution, exp cannot overflow); the denominator is accumulated
    on the vector engine and reduced across partitions via ones-matmuls.
  - 1/rowsum is computed as exp(-ln(rowsum)) on the scalar engine (ln
    and exp share one ACT table set; DVE reciprocal is ~8 cycles/elem).
  - Normalization is fused into the AV-PSUM evacuation; merged per-head
    outputs feed the output projection as its stationary operand.
  - Emission is software-pipelined: scores lead one k-block so the PE
    never sits behind the exp stream in queue order, and out-projection
    matmuls of a finished q-block are dripped into the next block's
    PE stream as filler (keeps the PE busy so the HAM clock gate stays
    at 2.4 GHz).
"""

import numpy as np
import ml_dtypes

import bass_rust
import concourse.bass as bass
import concourse.mybir as mybir
import concourse.tile as tile
from concourse.vector_clock import ScopedClock
from concourse.bass_utils import run_bass_kernel_spmd

B, N, D = 2, 2048, 512
H, DH = 8, 64
SCALE = (2 * DH) ** -0.5
NCORES = 8
HPC = 2              # heads per core
CW = HPC * DH        # 128: per-core slice width of the inner dim
DC = D // 128        # 4 contraction chunks for the projections
NKB = N // 128       # 16 key blocks
QB = 1024            # q-block width for the attention inner loop
NQB = N // QB        # 2
BF16 = ml_dtypes.bfloat16

_WAIT_LIMIT = 1  # this container's walrus rejects multiple sync waits per instruction


def _split_sync_waits(nc):
    """Hoist excess semaphore waits onto same-engine NOPs inserted right
    before the over-budget instruction ("Too many sync wait commands")."""
    for f in nc.m.functions:
        for bb in f.blocks:
            insts = bb.instructions
            i = 0
            while i < len(insts):
                inst = insts[i]
                si = inst.sync_info
                if si is None:
                    i += 1
                    continue
                waits = list(si.on_wait)
                sem_waits = [w for w in waits if w.sync_type == "semaphore"]
                other = [w for w in waits if w.sync_type != "semaphore"]
                budget = _WAIT_LIMIT - len(other)
                if len(sem_waits) <= budget:
                    i += 1
                    continue
                keep = sem_waits[-budget:] if budget > 0 else []
                extra = sem_waits[:-budget] if budget > 0 else sem_waits
                for j in range(0, len(extra), _WAIT_LIMIT):
                    nop = mybir.InstNoOp(
                        name=f"I-{nc.next_id()}",
                        engine=inst.engine,
                        bass_nofuse=True,
                        sync_info=mybir.SyncInfo(
                            on_wait=extra[j:j + _WAIT_LIMIT], on_update=[]
                        ),
                    )
                    insts.insert(i, nop)
                    i += 1
                si.on_wait = other + keep
                inst.sync_info = si
                i += 1


def _body(nc, tc):
    bf = mybir.dt.bfloat16
    f32 = mybir.dt.float32
    EXP = mybir.ActivationFunctionType.Exp
    LN = mybir.ActivationFunctionType.Ln

    x1T = nc.dram_tensor("x1T", [D, N], bf, kind="ExternalInput").ap()
    x2T = nc.dram_tensor("x2T", [D, N], bf, kind="ExternalInput").ap()
    # weights come pre-rearranged from the host as [p, dc*c] so the load
    # streams contiguous 1KB lines instead of 256B gather lines
    wq = [nc.dram_tensor(f"wq{s}", [128, DC * CW], bf, kind="ExternalInput").ap() for s in (1, 2)]
    wk = [nc.dram_tensor(f"wk{s}", [128, DC * CW], bf, kind="ExternalInput").ap() for s in (1, 2)]
    wv = [nc.dram_tensor(f"wv{s}", [128, DC * CW], bf, kind="ExternalInput").ap() for s in (1, 2)]
    wout = nc.dram_tensor("wout", [CW, D], bf, kind="ExternalInput").ap()
    ident = nc.dram_tensor("ident", [128, 128], bf, kind="ExternalInput").ap()
    out = nc.dram_tensor("out", [2 * N, D], bf, kind="ExternalOutput").ap()
    xT = [x1T, x2T]

    pools = []

    def mkpool(**kw):
        p = tc.alloc_tile_pool(**kw)
        pools.append(p)
        return p

    singles = mkpool(name="singles", bufs=1)
    spool = mkpool(name="spool", bufs=4, space="PSUM")      # 4x [128,512]f32 = 4 banks
    avpool = mkpool(name="avpool", bufs=2, space="PSUM")    # 2x [128,1024]f32 = 4 banks
    ptpool = mkpool(name="ptpool", bufs=8)
    accpool = mkpool(name="accpool", bufs=2)
    smallpool = mkpool(name="smallpool", bufs=3)
    stagepool = mkpool(name="stagepool", bufs=4)
    ostage = mkpool(name="ostage", bufs=8)

    # ---- resident inputs -------------------------------------------------
    # Load order matters for the PE start: qk weights, then x quarter 0,
    # then the rest, spread over the three DMA-trigger queues.
    dma_engines = [nc.sync, nc.scalar, nc.gpsimd]

    def load_w(ap, name, eng):
        t = singles.tile([128, DC, CW], bf, tag=name, name=name)
        eng.dma_start(out=t, in_=ap.rearrange("p (dc c) -> p dc c", dc=DC))
        return t

    x_sb = [[singles.tile([128, N], bf, tag=f"x{s}_{dc}", name=f"x{s}_{dc}")
             for dc in range(DC)] for s in range(2)]

    def load_x_half(half):
        n0, n1 = half * (N // 2), (half + 1) * (N // 2)
        for s in range(2):
            for dc in range(DC):
                eng = dma_engines[(s * DC + dc) % len(dma_engines)]
                eng.dma_start(out=x_sb[s][dc][:, n0:n1],
                              in_=xT[s][dc * 128:(dc + 1) * 128, n0:n1])

    load_x_half(0)
    wq_sb = [load_w(wq[s], f"wq{s}", dma_engines[s]) for s in range(2)]
    wk_sb = [load_w(wk[s], f"wk{s}", dma_engines[(2 + s) % 3]) for s in range(2)]
    wv_sb = [load_w(wv[s], f"wv{s}", dma_engines[s]) for s in range(2)]
    load_x_half(1)

    ones_mat = singles.tile([128, 128], bf, tag="ones", name="ones")
    nc.vector.memset(ones_mat, 1.0)
    ident_sb = singles.tile([128, 128], bf, tag="ident", name="ident")
    nc.gpsimd.dma_start(out=ident_sb, in_=ident)
    wout_sb = singles.tile([CW, D], bf, tag="wout", name="wout")
    nc.scalar.dma_start(out=wout_sb, in_=wout)

    # persistent layout targets
    qt = [singles.tile([128, N], bf, tag=f"qt{h}", name=f"qt{h}") for h in range(HPC)]
    kt = [singles.tile([128, N], bf, tag=f"kt{h}", name=f"kt{h}") for h in range(HPC)]
    vt_sb = [singles.tile([128, N], bf, tag=f"vt{s}", name=f"vt{s}") for s in range(2)]
    v_all = singles.tile([128, NKB, HPC, 2, DH], bf, tag="vall", name="vall")
    merged = [singles.tile([128, N], bf, tag=f"merged{s}", name=f"merged{s}") for s in range(2)]

    n_dma = [0]

    def next_dma():
        e = dma_engines[n_dma[0] % len(dma_engines)]
        n_dma[0] += 1
        return e

    # ---- projections, 512-wide psum units over both pools ----------------
    # spool gives four 1-bank slots during the projection phase (attention
    # hasn't started), so the PE never stalls on evacuation; evac copies
    # alternate scalar/vector.
    n_evac = [0]

    def evac(out_ap, in_ap, eng=None):
        if eng is None:
            eng = nc.scalar if n_evac[0] % 2 == 0 else nc.vector
            n_evac[0] += 1
        if eng is nc.scalar:
            nc.scalar.copy(out=out_ap, in_=in_ap)
        else:
            eng.tensor_copy(out=out_ap, in_=in_ap)

    def proj_qk_unit(nch, dst, w_sb, s, eng=None):
        """One (dst, stream) 1024-col chunk of the QK projection: M=128
        matmuls (both heads at once); the per-head stream-stacked layout
        is then assembled by SBUF->SBUF DMA."""
        c0 = nch * 1024
        st = stagepool.tile([128, 1024], bf, tag="qkstage", name="qkstage")
        for half in range(2):
            h0 = c0 + half * 512
            ps = spool.tile([128, 512], f32, tag="s", name="qkps")
            for dc in range(DC):
                nc.tensor.matmul(
                    ps,
                    lhsT=w_sb[s][:, dc, :],
                    rhs=x_sb[s][dc][:, h0:h0 + 512],
                    start=(dc == 0),
                    stop=(dc == DC - 1),
                )
            evac(st[:, half * 512:(half + 1) * 512], ps, eng)
        for h in range(HPC):
            next_dma().dma_start(
                out=dst[h][s * 64:(s + 1) * 64, c0:c0 + 1024],
                in_=st[h * 64:(h + 1) * 64, :],
            )

    def proj_qk_chunk(nch):
        for dst, w_sb in ((qt, wq_sb), (kt, wk_sb)):
            for s in range(2):
                proj_qk_unit(nch, dst, w_sb, s)

    def proj_vt_unit(nch, s, eng=None):
        c0 = nch * 1024
        for half in range(2):
            h0 = c0 + half * 512
            ps = spool.tile([128, 512], f32, tag="s", name="vtps")
            for dc in range(DC):
                nc.tensor.matmul(
                    ps,
                    lhsT=wv_sb[s][:, dc, :],
                    rhs=x_sb[s][dc][:, h0:h0 + 512],
                    start=(dc == 0),
                    stop=(dc == DC - 1),
                )
            evac(vt_sb[s][:, h0:h0 + 512], ps, eng)

    def proj_v_transpose(nch):
        """PE-transpose each key block of VT back into v_all natural
        layout [k, (kb, h, s, dh)]; both streams share one psum tile."""
        vps = avpool.tile([128, 2048], bf, tag="av", name="vps")
        kb0 = nch * 8
        for s in range(2):
            for i in range(8):
                nc.tensor.transpose(
                    vps[:, s * 1024 + i * 128:s * 1024 + (i + 1) * 128],
                    vt_sb[s][:, (kb0 + i) * 128:(kb0 + i + 1) * 128],
                    ident_sb,
                )
        for s in range(2):
            nc.vector.tensor_copy(
                out=v_all[:, kb0:kb0 + 8, :, s, :],
                in_=vps[:, s * 1024:(s + 1) * 1024].rearrange(
                    "p (i h d) -> p i h d", i=8, h=HPC),
            )

    def proj_v_chunk(nch, eng=None):
        for s in range(2):
            proj_vt_unit(nch, s, eng)
        proj_v_transpose(nch)

    # ---- out-projection ops (emitted as PE filler or at the tail) --------
    def op_emit(s, rb, cast_eng, alt_pool=False):
        if alt_pool:
            op = avpool.tile([128, 512], f32, tag="av", name="op")
        else:
            op = spool.tile([128, 512], f32, tag="s", name="op")
        nc.tensor.matmul(
            op,
            lhsT=merged[s][:, rb * 128:(rb + 1) * 128],
            rhs=wout_sb,
            start=True, stop=True,
        )
        ost = ostage.tile([128, 512], bf, tag="ost", name="ost")
        if cast_eng is nc.scalar:
            cast_eng.copy(out=ost, in_=op)
        else:
            cast_eng.tensor_copy(out=ost, in_=op)
        next_dma().dma_start(
            out=out[s * N + rb * 128:s * N + (rb + 1) * 128, :],
            in_=ost,
        )

    def outproj_filler(qb):
        """The 16 out-projection row blocks of q-block qb as a list of
        thunks, to be dripped into a later attention block's emission."""
        return [
            (lambda s=s, rb=qb * (QB // 128) + j: op_emit(s, rb, nc.vector))
            for j in range(QB // 128) for s in range(2)
        ]

    # ---- attention -------------------------------------------------------
    def score_mm(h, q0, kb):
        tiles = []
        for qh in range(2):
            st = spool.tile([128, 512], f32, tag="s", name="s")
            nc.tensor.matmul(
                st,
                lhsT=kt[h][:, kb * 128:(kb + 1) * 128],
                rhs=qt[h][:, q0 + qh * 512:q0 + (qh + 1) * 512],
                start=True, stop=True,
            )
            tiles.append(st)
        return tiles

    def attn_block(qb, h, filler, inserts=None, warm_tail=False):
        """Score/exp/AV loop; scores lead one kb; filler thunks are
        popped every other kb (out-projection of an earlier q-block);
        inserts maps kb -> thunk emitted after that kb (late projection
        work dripped into the PE stream while exp paces the loop).
        The denominator add-chain stops two k-blocks early: the last two
        pt tiles enter the ones-matmul reduction directly via PSUM
        accumulation, shortening the tail latency before ln/exp."""
        q0 = qb * QB
        av = avpool.tile([128, 1024], f32, tag="av", name="av")
        acc = accpool.tile([128, 1024], bf, tag="acc", name="acc")
        bcs = []
        st = score_mm(h, q0, 0)
        for kb in range(NKB):
            if inserts and kb in inserts:
                inserts[kb]()
            pt = ptpool.tile([128, 1024], bf, tag="pt", name="pt")
            for qh in range(2):
                nc.scalar.activation(
                    out=pt[:, qh * 512:(qh + 1) * 512], in_=st[qh],
                    func=EXP, scale=SCALE,
                )
            if kb < NKB - 1:
                st = score_mm(h, q0, kb + 1)
            for qh in range(2):
                nc.tensor.matmul(
                    av[:, qh * 512:(qh + 1) * 512],
                    lhsT=v_all[:, kb, h, :, :],
                    rhs=pt[:, qh * 512:(qh + 1) * 512],
                    start=(kb == 0), stop=(kb == NKB - 1),
                )
            if kb == NKB - 2:
                # partition-reduce acc (kb 0..13) and this pt into bc
                for qh in range(2):
                    sl = slice(qh * 512, (qh + 1) * 512)
                    bc = spool.tile([128, 512], f32, tag="s", name="bc")
                    nc.tensor.matmul(bc, lhsT=ones_mat, rhs=acc[:, sl],
                                     start=True, stop=False)
                    nc.tensor.matmul(bc, lhsT=ones_mat, rhs=pt[:, sl],
                                     start=False, stop=False)
                    bcs.append(bc)
            elif kb == NKB - 1:
                for qh in range(2):
                    sl = slice(qh * 512, (qh + 1) * 512)
                    nc.tensor.matmul(bcs[qh], lhsT=ones_mat, rhs=pt[:, sl],
                                     start=False, stop=True)
            if filler and (kb % 2 == 1 or kb == 2) and kb < NKB - 1:
                filler.pop(0)()
            if kb == 0:
                nc.vector.tensor_copy(out=acc, in_=pt)
            elif kb < NKB - 2:
                nc.vector.tensor_add(out=acc, in0=acc, in1=pt)

        if warm_tail:
            # keep the PE busy across the fold-wait so the HAM clock gate
            # doesn't throttle it right before the tail out-projection
            for i in range(4):
                dmy = spool.tile([128, 512], f32, tag="s", name="dmy")
                nc.tensor.matmul(dmy, lhsT=warm_sb[:, 0:128], rhs=warm_sb,
                                 start=True, stop=True)

        # 1/D = exp(-ln(D)); normalization folded into AV evacuation
        rcp = smallpool.tile([128, 1024], f32, tag="rcp", name="rcp")
        for qh in range(2):
            sl = slice(qh * 512, (qh + 1) * 512)
            lnd = smallpool.tile([128, 512], f32, tag="lnd", name="lnd")
            nc.scalar.activation(out=lnd, in_=bcs[qh], func=LN)
            nc.scalar.activation(out=rcp[:, sl], in_=lnd, func=EXP, scale=-1.0)
        for s in range(2):
            for qh in range(2):
                sl = slice(qh * 512, (qh + 1) * 512)
                nc.vector.tensor_mul(
                    out=merged[s][h * 64:(h + 1) * 64,
                                  q0 + qh * 512:q0 + (qh + 1) * 512],
                    in0=av[s * 64:(s + 1) * 64, sl],
                    in1=rcp[s * 64:(s + 1) * 64, sl],
                )

    # ---- emission schedule ----------------------------------------------
    # Pre-warm the PE while the input DMAs are in flight: the HAM clock
    # gate starts at 1.2 GHz and needs sustained near-full PE activity to
    # release to 2.4 GHz. fp32 matmuls run 4 cycles/row (~94% duty per
    # instruction) and depend on nothing but a memset tile.
    warm_sb = singles.tile([128, 512], f32, tag="warm", name="warm")
    nc.vector.memset(warm_sb, 0.5)
    for _ in range(6):
        dmy = spool.tile([128, 512], f32, tag="s", name="dmy")
        nc.tensor.matmul(dmy, lhsT=warm_sb[:, 0:128], rhs=warm_sb,
                         start=True, stop=True)

    # Chunk-0 projections up front; chunk-1 projection units are dripped
    # into the first attention block's kb stream (their outputs are only
    # needed from kb 8 on). Evacs of dripped units go to the vector
    # engine so the exp stream on ACT is not disturbed.
    proj_qk_chunk(0)
    proj_v_chunk(0)

    inserts00 = {
        1: lambda: proj_qk_unit(1, kt, wk_sb, 0, nc.vector),
        2: lambda: proj_qk_unit(1, kt, wk_sb, 1, nc.vector),
        3: lambda: proj_vt_unit(1, 0, nc.vector),
        4: lambda: proj_vt_unit(1, 1, nc.vector),
        5: lambda: proj_v_transpose(1),
    }
    inserts01 = {
        1: lambda: proj_qk_unit(1, qt, wq_sb, 0, nc.vector),
        2: lambda: proj_qk_unit(1, qt, wq_sb, 1, nc.vector),
    }
    attn_block(0, 0, None, inserts00)
    # rotation spacer: keeps the next av accumulator off the slot of the
    # (not yet folded) previous one, so its first write doesn't stall
    spacer = avpool.tile([128, 1], f32, tag="av", name="spacer")
    nc.vector.memset(spacer, 0.0)
    attn_block(0, 1, None, inserts01)
    fill0 = outproj_filler(0)
    attn_block(1, 0, fill0)
    attn_block(1, 1, fill0, warm_tail=True)
    assert not fill0
    # tail out-projection: casts alternate vector/scalar (both idle now);
    # stream-major order so s=0 rows start as soon as their folds land
    for s in range(2):
        for j in range(QB // 128):
            op_emit(s, (QB // 128) + j, nc.vector if (j + s) % 2 else nc.scalar,
                    alt_pool=j % 2 == 0)

    for p in reversed(pools):
        p.release()


_NC_CACHE = None


def _build():
    global _NC_CACHE
    if _NC_CACHE is None:
        nc = bass.Bass("TRN2", target_bir_lowering=False, debug=False)
        with tile.TileContext(nc) as tc:
            _body(nc, tc)
        _split_sync_waits(nc)
        _NC_CACHE = nc
    return _NC_CACHE


def _prep_in_maps(x1, x2, W_qkv1, W_qkv2, W_out):
    x1 = np.asarray(x1, np.float32)
    x2 = np.asarray(x2, np.float32)
    W1 = np.asarray(W_qkv1, np.float32).astype(BF16)
    W2 = np.asarray(W_qkv2, np.float32).astype(BF16)
    Wo = np.asarray(W_out, np.float32).astype(BF16)
    ident = np.eye(128, dtype=BF16)
    xT = [
        [np.ascontiguousarray(x[b].T).astype(BF16) for b in range(B)]
        for x in (x1, x2)
    ]

    def wprep(w):
        # [D, CW] -> [p, dc*c]: contiguous per-partition lines for the DMA
        return np.ascontiguousarray(
            w.reshape(DC, 128, CW).transpose(1, 0, 2).reshape(128, DC * CW))

    in_maps = []
    for c in range(NCORES):
        b, hg = divmod(c, NCORES // B)
        cs = slice(hg * CW, (hg + 1) * CW)
        in_maps.append({
            "x1T": xT[0][b],
            "x2T": xT[1][b],
            "wq1": wprep(W1[:, 0:D][:, cs]),
            "wq2": wprep(W2[:, 0:D][:, cs]),
            "wk1": wprep(W1[:, D:2 * D][:, cs]),
            "wk2": wprep(W2[:, D:2 * D][:, cs]),
            "wv1": wprep(W1[:, 2 * D:3 * D][:, cs]),
            "wv2": wprep(W2[:, 2 * D:3 * D][:, cs]),
            "wout": np.ascontiguousarray(Wo[cs, :]),
            "ident": ident,
        })
    return in_maps


def _run(inputs, **spmd_kwargs):
    nc = _build()
    in_maps = _prep_in_maps(
        inputs["x1"], inputs["x2"], inputs["W_qkv1"], inputs["W_qkv2"],
        inputs["W_out"],
    )
    res = run_bass_kernel_spmd(nc, in_maps, core_ids=list(range(NCORES)),
                               **spmd_kwargs)
    b_out = np.asarray(inputs["b_out"], np.float32)
    gpc = NCORES // B
    full = np.zeros((B, 2 * N, D), np.float32)
    for c in range(NCORES):
        full[c // gpc] += res.results[c]["out"].astype(np.float32)
    full += b_out
    return full, res


def kernel(**inputs):
    full, _ = _run(inputs)
    return full
